# revision 7
# baseline (speedup 1.0000x reference)
"""Trainium2 Bass kernel for the depth-weight (patchmatch confidence) module.

Contract: kernel(**inputs) takes FULL unsharded inputs (numpy), returns the
FULL [2,48,512,640] float32 output. Internally shards across 8 NeuronCores:
core c handles batch c//4, rows (c%4)*128 .. +128.

Device layout: 128 SBUF partitions = 8x16 grid of spatial blocks, each block
16 rows x 40 cols of the core's 128x640 strip. Each partition's free dim holds
a halo'd 24x48 window of x (normalized inverse depth) so all 9-neighbor
shifted reads are free-dim offsets (cross-partition reads are illegal on DVE).
Host reflect-pads depth by 4 so every DMA window is plain affine.
"""
import sys

sys.path.insert(0, "/opt/trn_rl_repo")
import numpy as np

B, D, H, W = 2, 48, 512, 640
NB = 9          # evaluate_neighbors
S2 = 2 * NB     # offset channels
STRIP = H // 4  # 128 rows per core strip
HB, WB = 8, 16  # partition grid: 8 h-blocks x 16 w-blocks
BH, BW = 16, 40  # block size (output pixels per partition)
PAD = 4
DC = 4          # d-planes per chunk
NCHUNK = D // DC
DPH, DPW = STRIP + 2 * PAD, W + 2 * PAD  # 136 x 648 per-core padded strip

_CACHE = {}


def _terms():
    t = []
    for s in range(NB):
        t.append((s, 4 * (s // 3), 4 * (s % 3)))              # far (xp1, pad 4)
        t.append((NB + s, 2 * (s // 3) + 2, 2 * (s % 3) + 2))  # near (xp, pad 2)
    return t


def _build(pis):
    import concourse.bass as bass
    import concourse.bacc as bacc
    import concourse.tile as tile
    from concourse import mybir

    f32 = mybir.dt.float32
    AF = mybir.ActivationFunctionType

    nc = bacc.Bacc("TRN2", target_bir_lowering=False, debug=False, num_devices=8)
    dpad_t = nc.dram_tensor("dpad", [D, DPH, DPW], f32, kind="ExternalInput")
    off_t = nc.dram_tensor("off", [S2, STRIP, W], f32, kind="ExternalInput")
    cc_t = nc.dram_tensor("cc", [128, 2], f32, kind="ExternalInput")
    out_t = nc.dram_tensor("out", [D, STRIP, W], f32, kind="ExternalOutput")

    d_str, h_str = DPH * DPW, DPW          # dpad strides (elements)
    os_str, oh_str = STRIP * W, W          # off/out strides

    terms = _terms()
    clipv = float(4.0 * pis)
    sig_scale = float(-2.0 / pis)

    with tile.TileContext(nc) as tc:
        with (
            tc.tile_pool(name="const", bufs=1) as cpool,
            tc.tile_pool(name="x", bufs=2) as xpool,
            tc.tile_pool(name="acc", bufs=2) as apool,
            tc.tile_pool(name="prod", bufs=2) as ppool,
            tc.tile_pool(name="res", bufs=2) as rpool,
        ):
            cct = cpool.tile([128, 2], f32)
            nc.sync.dma_start(cct[:], cc_t.ap())
            bias4 = cpool.tile([128, 1], f32)
            nc.gpsimd.memset(bias4[:], 4.0)
            Ot = cpool.tile([128, S2, BH, BW], f32)
            for hb in range(HB):
                for s in range(S2):
                    src = bass.AP(
                        off_t,
                        s * os_str + hb * BH * oh_str,
                        [[BW, WB], [oh_str, BH], [1, BW]],
                    )
                    eng = nc.sync if (hb + s) % 2 == 0 else nc.scalar
                    eng.dma_start(Ot[hb * BH : (hb + 1) * BH, s], src)
            # fold the 0.5 factor into the weights once
            nc.vector.tensor_scalar_mul(Ot[:], Ot[:], 0.5)

            for chunk in range(NCHUNK):
                d0 = chunk * DC
                X = xpool.tile([128, DC, BH + 2 * PAD, BW + 2 * PAD], f32)
                for hb in range(HB):
                    for d in range(DC):
                        src = bass.AP(
                            dpad_t,
                            (d0 + d) * d_str + hb * BH * h_str,
                            [[BW, WB], [h_str, BH + 2 * PAD], [1, BW + 2 * PAD]],
                        )
                        eng = nc.sync if (hb + d) % 2 == 0 else nc.scalar
                        eng.dma_start(X[hb * BH : (hb + 1) * BH, d], src)
                # x = c1 / depth + c0 (normalized inverse depth), incl. halo
                nc.vector.reciprocal(X[:], X[:])
                nc.scalar.activation(
                    X[:], X[:], AF.Identity, bias=cct[:, 1:2], scale=cct[:, 0:1]
                )

                A = apool.tile([128, DC, BH, BW], f32)
                P = ppool.tile([128, DC, BH, BW], f32)
                for i, (ch, ho, wo) in enumerate(terms):
                    xs = X[:, :, ho : ho + BH, wo : wo + BW]
                    ob = Ot[:, ch].unsqueeze(1).broadcast_to([128, DC, BH, BW])
                    if i == 0:
                        nc.vector.tensor_mul(A[:], xs, ob)
                    else:
                        nc.vector.tensor_mul(P[:], xs, ob)
                        nc.vector.tensor_add(A[:], A[:], P[:])

                R = rpool.tile([128, DC, BH, BW], f32)
                xc = X[:, :, PAD : PAD + BH, PAD : PAD + BW]
                nc.vector.tensor_sub(A[:], A[:], xc)
                nc.scalar.activation(A[:], A[:], AF.Abs)
                nc.vector.tensor_scalar_min(A[:], A[:], clipv)
                nc.scalar.activation(R[:], A[:], AF.Sigmoid, bias=bias4[:], scale=sig_scale)

                for hb in range(HB):
                    for d in range(DC):
                        dst = bass.AP(
                            out_t,
                            (d0 + d) * os_str + hb * BH * oh_str,
                            [[BW, WB], [oh_str, BH], [1, BW]],
                        )
                        nc.gpsimd.dma_start(dst, R[hb * BH : (hb + 1) * BH, d])

    nc.compile()
    return nc


def kernel(depth_sample, depth_min, depth_max, offset,
           patchmatch_interval_scale, evaluate_neighbors):
    from concourse.bass_utils import run_bass_kernel_spmd

    depth_sample = np.asarray(depth_sample, dtype=np.float32)
    offset = np.asarray(offset, dtype=np.float32)
    depth_min = np.asarray(depth_min, dtype=np.float32)
    depth_max = np.asarray(depth_max, dtype=np.float32)
    pis = float(np.asarray(patchmatch_interval_scale))
    assert int(np.asarray(evaluate_neighbors)) == NB
    assert depth_sample.shape == (B, D, H, W) and offset.shape == (B, S2, H, W)

    key = ("v1", pis)
    if key not in _CACHE:
        _CACHE[key] = _build(pis)
    nc = _CACHE[key]

    dpad = np.pad(depth_sample, ((0, 0), (0, 0), (PAD, PAD), (PAD, PAD)), mode="reflect")
    in_maps = []
    for c in range(8):
        b, strip = c // 4, c % 4
        h0 = strip * STRIP
        inv_min = 1.0 / np.float64(depth_min[b])
        inv_max = 1.0 / np.float64(depth_max[b])
        c1 = np.float32(1.0 / (inv_min - inv_max))
        c0 = np.float32(-inv_max / (inv_min - inv_max))
        in_maps.append({
            "dpad": np.ascontiguousarray(dpad[b, :, h0 : h0 + DPH, :]),
            "off": np.ascontiguousarray(offset[b, :, h0 : h0 + STRIP, :]),
            "cc": np.tile(np.array([[c1, c0]], np.float32), (128, 1)),
        })

    try:
        res = run_bass_kernel_spmd(nc, in_maps, list(range(8)))
    except Exception:
        # transient device wedge (e.g. NRT_EXEC_UNIT_UNRECOVERABLE) — retry once
        res = run_bass_kernel_spmd(nc, in_maps, list(range(8)))
    out = np.empty((B, D, H, W), np.float32)
    for c in range(8):
        b, strip = c // 4, c % 4
        out[b, :, strip * STRIP : (strip + 1) * STRIP, :] = res.results[c]["out"]
    return out


# revision 8
# speedup vs baseline: 1.0675x; 1.0675x over previous
"""Trainium2 Bass kernel for the depth-weight (patchmatch confidence) module.

Contract: kernel(**inputs) takes FULL unsharded inputs (numpy), returns the
FULL [2,48,512,640] float32 output. Internally shards across 8 NeuronCores:
core c handles batch c//4, rows (c%4)*128 .. +128.

Device layout: 128 SBUF partitions = 8x16 grid of spatial blocks, each block
16 rows x 40 cols of the core's 128x640 strip. Each partition's free dim holds
a halo'd 24x48 window of x (normalized inverse depth) so all 9-neighbor
shifted reads are free-dim offsets (cross-partition reads are illegal on DVE).
Host reflect-pads depth by 4 so every DMA window is plain affine; the 0.5
neighbor-average factor and an fp16 cast are folded into the offset staging.

Numerics: x and weights cast to fp16 for the 18 products (DVE 2x mode), pair
sums in fp16, accumulation across the 9 pairs in fp32. Measured end-to-end
absmax-relative error vs the fp32 reference: 5.9e-3.
"""
import sys

sys.path.insert(0, "/opt/trn_rl_repo")
import numpy as np

B, D, H, W = 2, 48, 512, 640
NB = 9          # evaluate_neighbors
S2 = 2 * NB     # offset channels
STRIP = H // 4  # 128 rows per core strip
HB, WB = 8, 16  # partition grid: 8 h-blocks x 16 w-blocks
BH, BW = 16, 40  # block size (output pixels per partition)
PAD = 4
DC = 4          # d-planes per chunk
NCHUNK = D // DC
DPH, DPW = STRIP + 2 * PAD, W + 2 * PAD  # 136 x 648 per-core padded strip

_CACHE = {}


def _terms():
    t = []
    for s in range(NB):
        t.append((s, 4 * (s // 3), 4 * (s % 3)))              # far (xp1, pad 4)
        t.append((NB + s, 2 * (s // 3) + 2, 2 * (s % 3) + 2))  # near (xp, pad 2)
    return t


def _build(pis):
    import concourse.bass as bass
    import concourse.bacc as bacc
    import concourse.tile as tile
    from concourse import mybir

    f32 = mybir.dt.float32
    f16 = mybir.dt.float16
    AF = mybir.ActivationFunctionType

    nc = bacc.Bacc("TRN2", target_bir_lowering=False, debug=False, num_devices=8)
    dpad_t = nc.dram_tensor("dpad", [D, DPH, DPW], f32, kind="ExternalInput")
    off_t = nc.dram_tensor("off", [S2, STRIP, W], f16, kind="ExternalInput")
    cc_t = nc.dram_tensor("cc", [128, 2], f32, kind="ExternalInput")
    out_t = nc.dram_tensor("out", [D, STRIP, W], f32, kind="ExternalOutput")

    d_str, h_str = DPH * DPW, DPW          # dpad strides (elements)
    os_str, oh_str = STRIP * W, W          # off/out strides

    terms = _terms()
    clipv = float(4.0 * pis)
    sig_scale = float(-2.0 / pis)

    with tile.TileContext(nc) as tc:
        with (
            tc.tile_pool(name="const", bufs=1) as cpool,
            tc.tile_pool(name="x", bufs=2) as xpool,
            tc.tile_pool(name="x16", bufs=2) as x16pool,
            tc.tile_pool(name="acc", bufs=2) as apool,
            tc.tile_pool(name="pa", bufs=2) as papool,
            tc.tile_pool(name="pb", bufs=2) as pbpool,
            tc.tile_pool(name="res", bufs=2) as rpool,
        ):
            cct = cpool.tile([128, 2], f32)
            nc.sync.dma_start(cct[:], cc_t.ap())
            bias4 = cpool.tile([128, 1], f32)
            nc.gpsimd.memset(bias4[:], 4.0)

            # offsets (already 0.5-scaled fp16 on host) in block layout
            Ot = cpool.tile([128, S2, BH, BW], f16)
            for hb in range(HB):
                for s in range(S2):
                    src = bass.AP(
                        off_t,
                        s * os_str + hb * BH * oh_str,
                        [[BW, WB], [oh_str, BH], [1, BW]],
                    )
                    eng = nc.sync if (hb + s) % 2 == 0 else nc.scalar
                    eng.dma_start(Ot[hb * BH : (hb + 1) * BH, s], src)

            for chunk in range(NCHUNK):
                d0 = chunk * DC
                X = xpool.tile([128, DC, BH + 2 * PAD, BW + 2 * PAD], f32)
                for hb in range(HB):
                    for d in range(DC):
                        src = bass.AP(
                            dpad_t,
                            (d0 + d) * d_str + hb * BH * h_str,
                            [[BW, WB], [h_str, BH + 2 * PAD], [1, BW + 2 * PAD]],
                        )
                        eng = nc.sync if (hb + d) % 2 == 0 else nc.scalar
                        eng.dma_start(X[hb * BH : (hb + 1) * BH, d], src)
                # x = c1 / depth + c0 (normalized inverse depth), incl. halo
                nc.vector.reciprocal(X[:], X[:])
                X16 = x16pool.tile([128, DC, BH + 2 * PAD, BW + 2 * PAD], f16)
                nc.scalar.activation(
                    X16[:], X[:], AF.Identity, bias=cct[:, 1:2], scale=cct[:, 0:1]
                )

                A = apool.tile([128, DC, BH, BW], f32)
                Pa = papool.tile([128, DC, BH, BW], f16)
                Pb = pbpool.tile([128, DC, BH, BW], f16)
                for i in range(NB):
                    ch1, h1, w1 = terms[2 * i]
                    ch2, h2, w2 = terms[2 * i + 1]
                    xs1 = X16[:, :, h1 : h1 + BH, w1 : w1 + BW]
                    xs2 = X16[:, :, h2 : h2 + BH, w2 : w2 + BW]
                    ob1 = Ot[:, ch1].unsqueeze(1).broadcast_to([128, DC, BH, BW])
                    ob2 = Ot[:, ch2].unsqueeze(1).broadcast_to([128, DC, BH, BW])
                    nc.vector.tensor_mul(Pa[:], xs1, ob1)
                    nc.vector.tensor_mul(Pb[:], xs2, ob2)
                    if i == 0:
                        nc.vector.tensor_add(A[:], Pa[:], Pb[:])
                    else:
                        nc.vector.tensor_add(Pa[:], Pa[:], Pb[:])
                        nc.vector.tensor_add(A[:], A[:], Pa[:])

                R = rpool.tile([128, DC, BH, BW], f32)
                xc = X16[:, :, PAD : PAD + BH, PAD : PAD + BW]
                nc.vector.tensor_sub(A[:], A[:], xc)
                nc.scalar.activation(A[:], A[:], AF.Abs)
                nc.vector.tensor_scalar_min(A[:], A[:], clipv)
                nc.scalar.activation(R[:], A[:], AF.Sigmoid, bias=bias4[:], scale=sig_scale)

                for hb in range(HB):
                    for d in range(DC):
                        dst = bass.AP(
                            out_t,
                            (d0 + d) * os_str + hb * BH * oh_str,
                            [[BW, WB], [oh_str, BH], [1, BW]],
                        )
                        nc.gpsimd.dma_start(dst, R[hb * BH : (hb + 1) * BH, d])

    nc.compile()
    return nc


def kernel(depth_sample, depth_min, depth_max, offset,
           patchmatch_interval_scale, evaluate_neighbors):
    from concourse.bass_utils import run_bass_kernel_spmd

    depth_sample = np.asarray(depth_sample, dtype=np.float32)
    offset = np.asarray(offset, dtype=np.float32)
    depth_min = np.asarray(depth_min, dtype=np.float32)
    depth_max = np.asarray(depth_max, dtype=np.float32)
    pis = float(np.asarray(patchmatch_interval_scale))
    assert int(np.asarray(evaluate_neighbors)) == NB
    assert depth_sample.shape == (B, D, H, W) and offset.shape == (B, S2, H, W)

    key = ("v2", pis)
    if key not in _CACHE:
        _CACHE[key] = _build(pis)
    nc = _CACHE[key]

    dpad = np.pad(depth_sample, ((0, 0), (0, 0), (PAD, PAD), (PAD, PAD)), mode="reflect")
    off16 = (offset * np.float32(0.5)).astype(np.float16)
    in_maps = []
    for c in range(8):
        b, strip = c // 4, c % 4
        h0 = strip * STRIP
        inv_min = 1.0 / np.float64(depth_min[b])
        inv_max = 1.0 / np.float64(depth_max[b])
        c1 = np.float32(1.0 / (inv_min - inv_max))
        c0 = np.float32(-inv_max / (inv_min - inv_max))
        in_maps.append({
            "dpad": np.ascontiguousarray(dpad[b, :, h0 : h0 + DPH, :]),
            "off": np.ascontiguousarray(off16[b, :, h0 : h0 + STRIP, :]),
            "cc": np.tile(np.array([[c1, c0]], np.float32), (128, 1)),
        })

    try:
        res = run_bass_kernel_spmd(nc, in_maps, list(range(8)))
    except Exception:
        # transient device wedge (e.g. NRT_EXEC_UNIT_UNRECOVERABLE) — retry once
        res = run_bass_kernel_spmd(nc, in_maps, list(range(8)))
    out = np.empty((B, D, H, W), np.float32)
    for c in range(8):
        b, strip = c // 4, c % 4
        out[b, :, strip * STRIP : (strip + 1) * STRIP, :] = res.results[c]["out"]
    return out
